# revision 1
# baseline (speedup 1.0000x reference)
"""Attentional Factorization Machine kernel for 8 Trainium2 NeuronCores.

Data-parallel over batch: 1024 rows -> 128 per core. Per core, all field-pair
products hp are built on DVE via a cyclic-delta enumeration (fp16, 2x mode),
the attention MLP runs on the PE (fp16 streams, fp32 PSUM), relu on ACT, and
per-pair scores / p_w-projections are accumulated per batch row with one-hot
stationary matmuls; softmax + combine happen on-chip in a [128, 780] layout.
"""
import sys
for _p in ("/opt/trn_rl_repo",):
    if _p not in sys.path:
        sys.path.insert(0, _p)

import numpy as np

import os
import numpy as np
import concourse.bass as bass
import concourse.bacc as bacc
import concourse.mybir as mybir
import concourse.tile as tile

F32 = mybir.dt.float32
F16 = mybir.dt.float16
AF = mybir.ActivationFunctionType
ALU = mybir.AluOpType
AXIS = mybir.AxisListType

FLD = 40
NDELTA = 20
P = 780
HALF = 390


def build(nc, B_c=128, blocks=(16, 16, 32, 32, 32)):
    assert B_c == 128 and sum(blocks) == 128
    assert all(nb % 8 == 0 for nb in blocks)

    xTa_d = nc.dram_tensor("xTa", [128, B_c, 60], F16, kind="ExternalInput").ap()
    xTb_d = nc.dram_tensor("xTb", [128, B_c, 60], F16, kind="ExternalInput").ap()
    wT_d = nc.dram_tensor("wT", [128, 128], F16, kind="ExternalInput").ap()
    bias_d = nc.dram_tensor("bias", [128, 1], F32, kind="ExternalInput").ap()
    Zh_d = nc.dram_tensor("Zh", [128, 64], F16, kind="ExternalInput").ap()
    Zg_d = nc.dram_tensor("Zg", [128, 64], F16, kind="ExternalInput").ap()
    pb_d = nc.dram_tensor("pb", [128, 1], F32, kind="ExternalInput").ap()
    out_d = nc.dram_tensor("out", [B_c, 1], F32, kind="ExternalOutput").ap()

    with tile.TileContext(nc) as tc:
        with (
            tc.tile_pool(name="const", bufs=1) as cpool,
            tc.tile_pool(name="hp", bufs=2) as hpool,
            tc.tile_pool(name="relu", bufs=4) as rpool,
            tc.tile_pool(name="awps", bufs=2, space="PSUM") as awpool,
            tc.tile_pool(name="accps", bufs=1, space="PSUM") as accpool,
        ):
            wT_s = cpool.tile([128, 128], F16, tag="wT")
            bias_s = cpool.tile([128, 1], F32, tag="bias")
            Zh_s = cpool.tile([128, 64], F16, tag="Zh")
            Zg_s = cpool.tile([128, 64], F16, tag="Zg")
            pb_s = cpool.tile([128, 1], F32, tag="pb")
            nc.sync.dma_start(wT_s[:], wT_d[:])
            nc.sync.dma_start(bias_s[:], bias_d[:])
            nc.sync.dma_start(Zh_s[:], Zh_d[:])
            nc.sync.dma_start(Zg_s[:], Zg_d[:])
            nc.sync.dma_start(pb_s[:], pb_d[:])

            xTa = cpool.tile([128, B_c, 60], F16, tag="xTa")
            xTb = cpool.tile([128, B_c, 60], F16, tag="xTb")
            sc_h0 = accpool.tile([128, 512], F32, tag="sc_h0")
            sc_h1 = accpool.tile([128, 512], F32, tag="sc_h1")
            g_h0 = accpool.tile([128, 512], F32, tag="g_h0")
            g_h1 = accpool.tile([128, 512], F32, tag="g_h1")
            sc_h = [sc_h0, sc_h1]
            g_h = [g_h0, g_h1]

            # depth-2 software pipeline across pairs:
            #   stage A: mm1 for newest pair
            #   stage B: relu for previous pair
            #   stage C: scores/g for the pair before that
            pending = []     # (hp3, kbs) awaiting mm1
            relu_q = []      # (hp3, kbs, aws) awaiting relu
            scg_q = []       # (hp3, kbs, relus) awaiting scores/g

            def do_mm1(item):
                hp3, kbs = item
                ks = [k for k, b in kbs]
                aws = []
                nmm = 0
                for k in ks:
                    aw = awpool.tile([128, 1024], F32, tag="aw")
                    for h in (0, 1):
                        bi = nc.tensor.matmul(
                            aw[:, 512 * h:512 * h + HALF],
                            wT_s[:],
                            hp3[:, k, h * HALF:(h + 1) * HALF],
                            start=True, stop=True,
                        )
                        if nmm > 0:
                            bi.ins.ldweights = False
                        nmm += 1
                    aws.append(aw)
                relu_q.append((hp3, kbs, aws))

            def do_relu(item):
                hp3, kbs, aws = item
                ks = [k for k, b in kbs]
                relus = []
                for k, aw in zip(ks, aws):
                    relu = rpool.tile([128, P], F16, tag="relu")
                    aw_v = aw[:].rearrange("a (u q) -> a u q", q=512)[:, :, 0:HALF]
                    relu_v = relu[:].rearrange("a (u q) -> a u q", q=HALF)
                    nc.scalar.activation(relu_v, aw_v, AF.Relu, bias=bias_s[:])
                    relus.append(relu)
                scg_q.append((hp3, kbs, relus))

            def do_scg(item):
                hp3, kbs, relus = item
                # interleave scores/g and alternate banks + col groups so
                # adjacent PE instructions share no tensor, bank or col group
                for u in (0, 1):
                    for ki, ((k, b), relu) in enumerate(zip(kbs, relus)):
                        h = u ^ (ki & 1)
                        j, mp = b // 32, b % 32
                        st = mp == 0
                        sp = mp == 31
                        nc.tensor.matmul(
                            sc_h[h][32 * j:32 * j + 32, 0:HALF],
                            Zh_s[:, 32 - mp:64 - mp],
                            relu[:, h * HALF:(h + 1) * HALF],
                            start=st, stop=sp,
                            tile_position=(0, 32 * j),
                            skip_group_check=True,
                        )
                        kg, bg = kbs[1 - ki]
                        hg = 1 - h
                        jg, mpg = bg // 32, bg % 32
                        nc.tensor.matmul(
                            g_h[hg][32 * jg:32 * jg + 32, 0:HALF],
                            Zg_s[:, 32 - mpg:64 - mpg],
                            hp3[:, kg, hg * HALF:(hg + 1) * HALF],
                            start=(mpg == 0), stop=(mpg == 31),
                            tile_position=(0, 32 * jg),
                            skip_group_check=True,
                        )

            def step():
                if pending:
                    do_mm1(pending.pop(0))
                if len(relu_q) >= 2:
                    do_relu(relu_q.pop(0))
                if len(scg_q) >= 2:
                    do_scg(scg_q.pop(0))

            def flush():
                while pending or relu_q or scg_q:
                    if pending:
                        do_mm1(pending.pop(0))
                    if relu_q:
                        do_relu(relu_q.pop(0))
                    if scg_q:
                        do_scg(scg_q.pop(0))

            grp_count = [0, 0, 0, 0]
            bs = 0
            NBMAX = max(blocks)
            for t, NB in enumerate(blocks):
                nc.sync.dma_start(xTa[:, bs:bs + NB, :],
                                  xTa_d[:, bs:bs + NB, :])
                nc.sync.dma_start(xTb[:, bs:bs + NB, :],
                                  xTb_d[:, bs:bs + NB, :])

                hp = hpool.tile([128, NBMAX * P], F16, tag="hp")
                hp3 = hp[:].rearrange("e (b q) -> e b q", q=P)

                for d in range(1, NDELTA + 1):
                    cnt = FLD if d < NDELTA else NDELTA
                    col0 = (d - 1) * FLD
                    # keep both operands 4B-aligned so DVE 2x_1P engages:
                    # even d reads xTa at offset d, odd d reads xTb at d-1
                    if d % 2 == 0:
                        in1 = xTa[:, bs:bs + NB, d:d + cnt]
                    else:
                        in1 = xTb[:, bs:bs + NB, d - 1:d - 1 + cnt]
                    nc.vector.tensor_mul(
                        hp3[:, 0:NB, col0:col0 + cnt],
                        xTa[:, bs:bs + NB, 0:cnt],
                        in1,
                    )

                kbs_all = []
                for k in range(NB):
                    j = k % 4
                    b = 32 * j + grp_count[j]
                    grp_count[j] += 1
                    kbs_all.append((k, b))
                for pi in range(0, NB, 2):
                    pending.append((hp3, kbs_all[pi:pi + 2]))
                    step()
                bs += NB

            flush()

            # ---- softmax tail ----
            exp_s = cpool.tile([128, P], F32, tag="exp_s")
            junk = cpool.tile([128, P], F32, tag="junk")
            negm = cpool.tile([128, 1], F32, tag="negm")
            denom = cpool.tile([128, 1], F32, tag="denom")
            rden = cpool.tile([128, 1], F32, tag="rden")
            numer = cpool.tile([128, 1], F32, tag="numer")
            outc = cpool.tile([128, 1], F32, tag="outc")

            negm2 = cpool.tile([128, 2], F32, tag="negm2")
            den2 = cpool.tile([128, 2], F32, tag="den2")
            num2 = cpool.tile([128, 2], F32, tag="num2")
            for h in (0, 1):
                nc.vector.tensor_reduce(negm2[:, h:h + 1], sc_h[h][:, 0:HALF],
                                        axis=AXIS.X, op=ALU.max)
            # overall max per b (as negative): negm = -max(m0, m1)
            nc.vector.tensor_reduce(negm[:], negm2[:], axis=AXIS.X,
                                    op=ALU.max, negate=True)
            for h in (0, 1):
                nc.scalar.activation(exp_s[:, h * HALF:(h + 1) * HALF],
                                     sc_h[h][:, 0:HALF], AF.Exp, bias=negm[:],
                                     accum_out=den2[:, h:h + 1])
                nc.vector.tensor_mul(junk[:, h * HALF:(h + 1) * HALF],
                                     exp_s[:, h * HALF:(h + 1) * HALF],
                                     g_h[h][:, 0:HALF])
            nc.vector.tensor_reduce(numer[:], junk[:], axis=AXIS.X, op=ALU.add)
            nc.vector.tensor_reduce(denom[:], den2[:], axis=AXIS.X, op=ALU.add)
            nc.vector.reciprocal(rden[:], denom[:])
            nc.vector.tensor_mul(outc[:], numer[:], rden[:])
            nc.vector.tensor_scalar_add(outc[:], outc[:], pb_s[:])
            nc.sync.dma_start(out_d[:], outc[:])

    nc.compile()
    return nc


def make_nc(B_c=128, blocks=(16, 16, 32, 32, 32)):
    nc = bacc.Bacc("TRN2", target_bir_lowering=False, debug=False)
    build(nc, B_c=B_c, blocks=blocks)
    return nc


def perm_for(B_c=128, blocks=(16, 16, 32, 32, 32)):
    """perm[slot] = global b stored at SBUF slot."""
    grp_count = [0, 0, 0, 0]
    perm = []
    for nb in blocks:
        for k in range(nb):
            j = k % 4
            perm.append(32 * j + grp_count[j])
            grp_count[j] += 1
    return np.array(perm, np.int64)


def host_prep_consts(attn_w_w, attn_w_b, attn_h_w, attn_h_b, attn_p_w, attn_p_b):
    wT = np.ascontiguousarray(attn_w_w.T).astype(np.float16)
    bias = attn_w_b.reshape(128, 1).astype(np.float32)
    Zh = np.zeros((128, 64), np.float16)
    Zh[:, 32] = attn_h_w[0].astype(np.float16)
    Zg = np.zeros((128, 64), np.float16)
    Zg[:, 32] = attn_p_w[0].astype(np.float16)
    pb = np.full((128, 1), np.float32(attn_p_b[0]), np.float32)
    return {"wT": wT, "bias": bias, "Zh": Zh, "Zg": Zg, "pb": pb}


def host_prep_x(x_slice, blocks=(16, 16, 32, 32, 32)):
    # [B_c, F, E] -> two pre-shifted fp16 copies [E, B_c(perm), 60]
    xT = x_slice.transpose(2, 0, 1).astype(np.float16)
    xT = xT[:, perm_for(x_slice.shape[0], blocks), :]
    B_c = x_slice.shape[0]
    xa = np.zeros((128, B_c, 60), np.float16)
    xa[:, :, 0:40] = xT
    xa[:, :, 40:60] = xT[:, :, 0:20]
    xb = np.zeros((128, B_c, 60), np.float16)
    xb[:, :, 0:59] = xa[:, :, 1:60]
    return np.ascontiguousarray(xa), np.ascontiguousarray(xb)




_NC_CACHE = {}
_BLOCKS = (8, 8, 16, 32, 32, 32)


def _get_nc():
    key = _BLOCKS
    if key not in _NC_CACHE:
        _NC_CACHE[key] = make_nc(B_c=128, blocks=key)
    return _NC_CACHE[key]


def kernel(x, attn_w_w, attn_w_b, attn_h_w, attn_h_b, attn_p_w, attn_p_b,
           _trace=False):
    from concourse.bass_utils import run_bass_kernel_spmd
    x = np.asarray(x, np.float32)
    consts = host_prep_consts(np.asarray(attn_w_w), np.asarray(attn_w_b),
                              np.asarray(attn_h_w), np.asarray(attn_h_b),
                              np.asarray(attn_p_w), np.asarray(attn_p_b))
    in_maps = []
    for c in range(8):
        m = dict(consts)
        m["xTa"], m["xTb"] = host_prep_x(x[128 * c:128 * (c + 1)],
                                         blocks=_BLOCKS)
        in_maps.append(m)
    nc = _get_nc()
    res = run_bass_kernel_spmd(nc, in_maps, list(range(8)), trace=_trace)
    out = np.concatenate([res.results[c]["out"][:, 0] for c in range(8)])
    if _trace:
        return out.astype(np.float32), res
    return out.astype(np.float32)

